# revision 19
# baseline (speedup 1.0000x reference)
"""BiESN2D on 8 TRN2 NeuronCores (Bass/Tile).

Reference computes 4 directional leaky-tanh ESN scans over a (8,128,128,64)
image batch: horizontal fwd/bwd over rows, vertical fwd/bwd over columns,
each with U=256 units, outputs concatenated to (8,128,128,1024).

Sharding: core = (scan-direction, batch-half).  Each of the 8 cores runs ONE
scan type over S=512 sequences (4 batches x 128 rows/cols), T=128 steps.

The recurrence is kept in z-space (pre-activation) with the leak folded into
exponentially-scaled weights, which removes BOTH the leaky blend and the
decay op from the per-step serial chain:
    z_{t+1} = 0.1*z_t + xk'_{t+1} + g_t @ Wr',   g_t = tanh(z_t)
with Wr' = 0.9*Wr and xk'_{t+1} = xk_{t+1} - 0.1*xk_t (x is packed with rows
0-63 = x_t, rows 64-127 = x_{t-1}, against a stationary [Wk; -0.1*Wk]).
One PSUM bank per chunk accumulates Z_tau = 10^tau * z over a W=6 step
window using 10^tau-prescaled fp16 weight copies (stationary weights are
reloaded every matmul anyway, so scaled copies are free), and the tanh reads
it with the free activation scale immediate:  g = tanh(10^-tau * Z).  At a
window boundary the carry 0.1*z is downcast to fp16 by VectorE (psum->sbuf)
and injected into the fresh bank with a single identity matmul.

Per step, per s-chunk (3 chains pipeline the PE -> ScalarE(tanh) path):
6 matmuls (2 K=128 xk' + 4 K=128 Wr', moving = previous tanh output) and one
ScalarE tanh (psum fp32 -> ring slot, sbuf fp16).  Every 4 steps one fully-
contiguous DMA ships 4 ring slots of g to DRAM; the x input streams in
16-step groups staggered through the loop (an upfront burst starves the
output DMAs and drops the HAM clock-gate mid-run).  The host runs the output
IIR w_t = 0.1*w_{t-1} + g_t (0.3% of the FLOPs) and scales h = 0.9*w in fp32.
An initial dep-free heater burst warms the PE HAM clock-gate to K=8/8.
All compute except PSUM accumulation is fp16.
"""

import numpy as np
from contextlib import ExitStack

import concourse.bass as bass
import concourse.mybir as mybir
import concourse.tile as tile
from concourse import bacc
from concourse.bass_utils import run_bass_kernel_spmd

# ---------------- problem constants (hardcoded per spec) ----------------
B, NH, NW, C = 8, 128, 128, 64
U = 256           # units per directional ESN cell
T = 128           # scan length
S = 512           # sequences per core (4 batches * 128)
LEAKY = 0.9
DECAY = 1.0 - LEAKY
N_CORES = 8

F16 = mybir.dt.float16
F32 = mybir.dt.float32

CHUNKS = (176, 176, 160)  # s-chunks; each <= 256 (two u'-tiles in one bank)
W = 8                     # scaling-window length
SCALE_OFF = 2             # scales run 10^-2..10^5: both ends fit fp16
RING = 24                 # g ring slots per chain
DMA_BATCH = 4             # t-steps per output DMA
XDMA_TGROUP = 8           # t-steps per input DMA chunk
XDMA_PRE = 3              # x t-groups DMA'd before the loop; rest staggered
HEAT_BURST = 20           # initial heater matmuls: ~2.7us warmup
HEAT_PER_STEP = 0         # dep-free filler matmuls per step


def build_program(chunks=CHUNKS, t_steps=T, s_total=S,
                  heat_burst=HEAT_BURST, heat_per_step=HEAT_PER_STEP):
    """Build the SPMD per-core Bass program (identical on all 8 cores)."""
    assert sum(chunks) == s_total and all(c <= 256 for c in chunks)
    assert t_steps % DMA_BATCH == 0 and RING % DMA_BATCH == 0

    nc = bacc.Bacc("TRN2", target_bir_lowering=False, debug=False,
                   num_devices=N_CORES)

    # x packed: rows 0-63 = x_t, rows 64-127 = x_{t-1} (zeros at t=0)
    x_d = nc.declare_dram_parameter("x", [128, t_steps * s_total], F16,
                                    isOutput=False)
    # wk[:, tau*256:+256] = 10^tau * [Wk; -0.1*Wk]   (128, W*256)
    wk_d = nc.declare_dram_parameter("wk", [128, W * 256], F16,
                                     isOutput=False)
    # wr[:, tau*256:+256] = 10^tau * 0.9*Wr          (256, W*256)
    wr_d = nc.declare_dram_parameter("wr", [256, W * 256], F16,
                                     isOutput=False)
    ident_d = nc.declare_dram_parameter("ident", [128, 128], F16,
                                        isOutput=False)
    # per-chain outputs: y{ch}[p, t, j*ncs + s] = g_t[u = j*128 + p, s]
    y_aps = [nc.declare_dram_parameter(f"y{ch}", [128, t_steps, 2 * ncs],
                                       F16, isOutput=True).ap()
             for ch, ncs in enumerate(chunks)]
    x_ap, wk_ap, wr_ap = x_d.ap(), wk_d.ap(), wr_d.ap()

    nch = len(chunks)
    offs = [sum(chunks[:i]) for i in range(nch)]
    Tanh = mybir.ActivationFunctionType.Tanh

    with ExitStack() as ctx:
        tc = ctx.enter_context(tile.TileContext(nc))
        const = ctx.enter_context(tc.tile_pool(name="const", bufs=1))
        x_sb = const.tile([128, t_steps * s_total], F16)
        wk_sb = const.tile([128, W * 256], F16)
        wr0_sb = const.tile([128, W * 256], F16)
        wr1_sb = const.tile([128, W * 256], F16)
        ident_sb = const.tile([128, 128], F16)
        junk = const.tile([128, 512], F16)
        # per-chain g rings: slot k at cols [k*2*ncs, (k+1)*2*ncs)
        rings = [const.tile([128, RING * 2 * chunks[ch]], F16,
                            name=f"gring{ch}") for ch in range(nch)]

        nc.sync.dma_start(wk_sb[:], wk_ap[:])
        nc.vector.memset(junk[:], 0.0)
        n_xgroups = (t_steps + XDMA_TGROUP - 1) // XDMA_TGROUP

        def x_dma(grp):
            lo = grp * XDMA_TGROUP * s_total
            hi = min((grp + 1) * XDMA_TGROUP, t_steps) * s_total
            nc.sync.dma_start(x_sb[:, lo:hi], x_ap[:, lo:hi])

        # first groups up front; the rest staggered inside the loop so the
        # output DMAs are not starved early (ring-reuse stalls drop the HAM
        # clock gate otherwise)
        for grp in range(min(XDMA_PRE, n_xgroups)):
            x_dma(grp)
        nc.sync.dma_start(wr0_sb[:], wr_ap[0:128, :])
        nc.sync.dma_start(wr1_sb[:], wr_ap[128:256, :])
        nc.sync.dma_start(ident_sb[:], ident_d.ap()[:])

        s16_pool = ctx.enter_context(tc.tile_pool(name="s16", bufs=2))
        # psum: chunks 0/1 get 3-deep rotation (6 banks); chunk 2 gets 2
        # (2 banks); the heater shares chunk 2's tag.
        ps_a = ctx.enter_context(tc.tile_pool(name="psa", bufs=3,
                                              space="PSUM"))
        ps_b = ctx.enter_context(tc.tile_pool(name="psb", bufs=2,
                                              space="PSUM"))

        def new_bank(ch, wi):
            pool = ps_a if ch < 2 else ps_b
            return pool.tile([128, 2 * chunks[ch]], F32, tag=f"ps{ch}",
                             name=f"ps{ch}_w{wi}")

        heat_ps = ps_b.tile([128, 2 * chunks[2]], F32, tag="ps2",
                            name="heat_ps")

        def heat(n, ps):
            for _ in range(n):
                nc.tensor.matmul(ps[:], junk[:, 0:128],
                                 junk[:, 0:2 * chunks[2]],
                                 start=True, stop=True)

        def gslot(ch, t):
            ncs = chunks[ch]
            k = t % RING
            return rings[ch][:, k * 2 * ncs:(k + 1) * 2 * ncs]

        # initial heater burst: warms HAM while x streams in
        heat(heat_burst, heat_ps)

        def x_mms(ps, ch, t, tau, start, stop_last=False):
            """xk'_t (10^tau-scaled): 2 K=128 matmuls vs 10^tau*[Wk;-.1Wk]."""
            ncs, off = chunks[ch], offs[ch]
            sl = slice(t * s_total + off, t * s_total + off + ncs)
            for j in range(2):
                nc.tensor.matmul(ps[:, j * ncs:(j + 1) * ncs],
                                 wk_sb[:, tau * 256 + j * 128:
                                       tau * 256 + (j + 1) * 128],
                                 x_sb[:, sl], start=start,
                                 stop=(stop_last and j == 1))

        def w_mms(ps, ch, g, tau, stop_last=True):
            """g @ (10^tau * Wr'): 4 K=128 matmuls."""
            ncs = chunks[ch]
            o = tau * 256
            nc.tensor.matmul(ps[:, 0:ncs], wr0_sb[:, o:o + 128],
                             g[:, 0:ncs], start=False, stop=False)
            nc.tensor.matmul(ps[:, 0:ncs], wr1_sb[:, o:o + 128],
                             g[:, ncs:2 * ncs], start=False, stop=False)
            nc.tensor.matmul(ps[:, ncs:2 * ncs], wr0_sb[:, o + 128:o + 256],
                             g[:, 0:ncs], start=False, stop=False)
            nc.tensor.matmul(ps[:, ncs:2 * ncs], wr1_sb[:, o + 128:o + 256],
                             g[:, ncs:2 * ncs], start=False, stop=stop_last)

        # prologue: window 0, tau=0: bank = xk_0 (x_{-1} rows are zero)
        cur = []
        for ch in range(nch):
            ps = new_bank(ch, 0)
            x_mms(ps, ch, 0, 0, start=True, stop_last=True)
            cur.append(ps)

        carry_scale = float(DECAY * 0.1 ** (W - 1) * 10.0 ** SCALE_OFF)
        for t in range(t_steps):
            heat(heat_per_step, heat_ps)
            if t > 0 and t % XDMA_TGROUP == 0:
                grp = t // XDMA_TGROUP + XDMA_PRE - 1
                if grp < n_xgroups:
                    x_dma(grp)
            tau = t % W
            boundary = t + 1 < t_steps and (t + 1) % W == 0
            nxt = []
            for ch in range(nch):
                ncs = chunks[ch]
                ps = cur[ch]
                g = gslot(ch, t)
                if boundary:
                    # carry 0.1*z to sbuf BEFORE the tanh: same-tile reads
                    # serialize in emission order, so this way the identity
                    # inject + x matmuls overlap the tanh instead of
                    # following it.
                    s16 = s16_pool.tile([128, 2 * ncs], F16,
                                        tag=f"s16_{ch}",
                                        name=f"s16_{ch}_{t}")
                    nc.vector.tensor_scalar_mul(s16[:], ps[:], carry_scale)
                nc.scalar.activation(g[:], ps[:], Tanh,
                                     scale=float(10.0 ** (SCALE_OFF - tau)))
                if t + 1 < t_steps:
                    if not boundary:
                        # continue accumulating in the same bank
                        tau2 = (t + 1) % W
                        x_mms(ps, ch, t + 1, tau2, start=False)
                        w_mms(ps, ch, g, tau2)
                        nxt.append(ps)
                    else:
                        ps2 = new_bank(ch, (t + 1) // W)
                        nc.tensor.matmul(ps2[:], ident_sb[:], s16[:],
                                         start=True, stop=False)
                        x_mms(ps2, ch, t + 1, 0, start=False)
                        w_mms(ps2, ch, g, 0)
                        nxt.append(ps2)

                last4 = t >= t_steps - 4
                batch = 2 if last4 else DMA_BATCH
                if t % batch == batch - 1:
                    # ring slots for the batch are contiguous, as is dst
                    k0 = (t - (batch - 1)) % RING
                    src = rings[ch][:, k0 * 2 * ncs:
                                    (k0 + batch) * 2 * ncs]
                    dst = y_aps[ch][:, t - (batch - 1):t + 1, :]
                    nc.sync.dma_start(dst, src)
            cur = nxt

    nc.compile()
    return nc


_PROGRAM = None

# test-harness knob: when trace=True, the BassKernelResults (with
# exec_time_ns from neuron-profile) is stashed in PROFILE["last"].
PROFILE = {"trace": False, "last": None}


def _get_program():
    global _PROGRAM
    if _PROGRAM is None:
        _PROGRAM = build_program()
    return _PROGRAM


def _pack_x(xs, t_steps, s_total):
    """(S, T, C) fp32 -> packed (128, T*S) fp16: rows 0-63 x_t, 64-127
    x_{t-1} (zeros at t=0)."""
    xt = np.ascontiguousarray(xs.transpose(2, 1, 0))      # (C, T, S)
    packed = np.zeros((128, t_steps * s_total), np.float16)
    pv = packed.reshape(2, 64, t_steps, s_total)
    pv[0] = xt
    pv[1, :, 1:] = xt[:, :-1]
    return packed.reshape(128, t_steps * s_total)


def kernel(**inputs):
    x = np.asarray(inputs["inputs"], np.float32)          # (8,128,128,64)
    wsets = [
        (np.asarray(inputs["h_fwd_k"]), np.asarray(inputs["h_fwd_r"])),
        (np.asarray(inputs["h_bwd_k"]), np.asarray(inputs["h_bwd_r"])),
        (np.asarray(inputs["v_fwd_k"]), np.asarray(inputs["v_fwd_r"])),
        (np.asarray(inputs["v_bwd_k"]), np.asarray(inputs["v_bwd_r"])),
    ]
    nc = _get_program()

    in_maps = []
    for core in range(N_CORES):
        scan, bhalf = core // 2, core % 2
        xb = x[bhalf * 4:(bhalf + 1) * 4]                 # (4, NH, NW, C)
        if scan >= 2:                                     # vertical: cols as seqs
            xb = xb.transpose(0, 2, 1, 3)                 # (4, NW, NH, C)
        xs = xb.reshape(S, T, C)
        if scan % 2 == 1:                                 # bwd: reverse time
            xs = np.ascontiguousarray(xs[:, ::-1])
        wk, wr = wsets[scan]
        wk2 = np.concatenate([wk, -DECAY * wk], axis=0)             # (128,256)
        wks = np.concatenate([(10.0 ** (tau - SCALE_OFF)) * wk2
                              for tau in range(W)],
                             axis=1).astype(np.float16)             # (128,W*256)
        wrs = np.concatenate([(10.0 ** (tau - SCALE_OFF)) * LEAKY * wr
                              for tau in range(W)],
                             axis=1).astype(np.float16)             # (256,W*256)
        in_maps.append({"x": _pack_x(xs, T, S), "wk": wks, "wr": wrs,
                        "ident": (10.0 ** -SCALE_OFF * np.eye(128)).astype(np.float16)})

    res = run_bass_kernel_spmd(nc, in_maps, list(range(N_CORES)),
                               trace=PROFILE["trace"])
    PROFILE["last"] = res
    results = res.results

    out = np.empty((B, NH, NW, 4 * U), np.float32)
    for core in range(N_CORES):
        scan, bhalf = core // 2, core % 2
        # concat per-chain outputs (128, T, 2*ncs) back to (p, t, j, s)
        g = np.concatenate(
            [results[core][f"y{ch}"].reshape(128, T, 2, ncs)
             for ch, ncs in enumerate(CHUNKS)], axis=3).astype(np.float32)
        # host IIR: w_t = 0.1*w_{t-1} + g_t;  h = 0.9*w
        h = np.empty_like(g)
        w = np.zeros((128, 2, S), np.float32)
        for t in range(T):
            w = DECAY * w + g[:, t]
            h[:, t] = w
        h *= LEAKY
        hs = h.transpose(3, 1, 2, 0).reshape(S, T, U)     # (s, t, u=(j,p))
        if scan % 2 == 1:
            hs = hs[:, ::-1]
        dst = out[bhalf * 4:(bhalf + 1) * 4, :, :, scan * U:(scan + 1) * U]
        if scan < 2:
            dst[:] = hs.reshape(4, NH, NW, U)
        else:
            dst[:] = hs.reshape(4, NW, NH, U).transpose(0, 2, 1, 3)
    return out


# revision 21
# speedup vs baseline: 1.1555x; 1.1555x over previous
"""BiESN2D on 8 TRN2 NeuronCores (Bass/Tile).

Reference computes 4 directional leaky-tanh ESN scans over a (8,128,128,64)
image batch: horizontal fwd/bwd over rows, vertical fwd/bwd over columns,
each with U=256 units, outputs concatenated to (8,128,128,1024).

Sharding: core = (scan-direction, batch-half).  Each of the 8 cores runs ONE
scan type over S=512 sequences (4 batches x 128 rows/cols), T=128 steps.

The recurrence is kept in z-space (pre-activation) with the leak folded into
exponentially-scaled weights, which removes BOTH the leaky blend and the
decay op from the per-step serial chain:
    z_{t+1} = 0.1*z_t + xk'_{t+1} + g_t @ Wr',   g_t = tanh(z_t)
with Wr' = 0.9*Wr and xk'_{t+1} = xk_{t+1} - 0.1*xk_t (x is packed with rows
0-63 = x_t, rows 64-127 = x_{t-1}, against a stationary [Wk; -0.1*Wk]).
One PSUM bank per chunk accumulates Z_tau = 10^tau * z over a W=6 step
window using 10^tau-prescaled fp16 weight copies (stationary weights are
reloaded every matmul anyway, so scaled copies are free), and the tanh reads
it with the free activation scale immediate:  g = tanh(10^-tau * Z).  At a
window boundary the carry 0.1*z is downcast to fp16 by VectorE (psum->sbuf)
and injected into the fresh bank with a single identity matmul.

Per step, per s-chunk (3 chains pipeline the PE -> ScalarE(tanh) path):
6 matmuls (2 K=128 xk' + 4 K=128 Wr', moving = previous tanh output) and one
ScalarE tanh (psum fp32 -> ring slot, sbuf fp16).  Every 4 steps one fully-
contiguous DMA ships 4 ring slots of g to DRAM; the x input streams in
16-step groups staggered through the loop (an upfront burst starves the
output DMAs and drops the HAM clock-gate mid-run).  The host runs the output
IIR w_t = 0.1*w_{t-1} + g_t (0.3% of the FLOPs) and scales h = 0.9*w in fp32.
An initial dep-free heater burst warms the PE HAM clock-gate to K=8/8.
All compute except PSUM accumulation is fp16.
"""

import numpy as np
from contextlib import ExitStack

import concourse.bass as bass
import concourse.mybir as mybir
import concourse.tile as tile
from concourse import bacc
from concourse.bass_utils import run_bass_kernel_spmd

# ---------------- problem constants (hardcoded per spec) ----------------
B, NH, NW, C = 8, 128, 128, 64
U = 256           # units per directional ESN cell
T = 128           # scan length
S = 512           # sequences per core (4 batches * 128)
LEAKY = 0.9
DECAY = 1.0 - LEAKY
N_CORES = 8

F16 = mybir.dt.float16
F32 = mybir.dt.float32

CHUNKS = (176, 176, 160)  # s-chunks; each <= 256 (two u'-tiles in one bank)
W = 6                     # scaling-window length (10^5 * wmax fits fp16)
RING = 24                 # g ring slots per chain
DMA_BATCH = 4             # t-steps per output DMA
XDMA_TGROUP = 8           # t-steps per input DMA chunk
XDMA_PRE = 3              # x t-groups DMA'd before the loop; rest staggered
HEAT_BURST = 28           # initial heater matmuls: ~3.7us warmup
HEAT_PER_STEP = 0         # dep-free filler matmuls per step


def build_program(chunks=CHUNKS, t_steps=T, s_total=S,
                  heat_burst=HEAT_BURST, heat_per_step=HEAT_PER_STEP):
    """Build the SPMD per-core Bass program (identical on all 8 cores)."""
    assert sum(chunks) == s_total and all(c <= 256 for c in chunks)
    assert t_steps % DMA_BATCH == 0 and RING % DMA_BATCH == 0

    nc = bacc.Bacc("TRN2", target_bir_lowering=False, debug=False,
                   num_devices=N_CORES)

    # x packed: rows 0-63 = x_t, rows 64-127 = x_{t-1} (zeros at t=0)
    x_d = nc.declare_dram_parameter("x", [128, t_steps * s_total], F16,
                                    isOutput=False)
    # wk[:, tau*256:+256] = 10^tau * [Wk; -0.1*Wk]   (128, W*256)
    wk_d = nc.declare_dram_parameter("wk", [128, W * 256], F16,
                                     isOutput=False)
    # wr[:, tau*256:+256] = 10^tau * 0.9*Wr          (256, W*256)
    wr_d = nc.declare_dram_parameter("wr", [256, W * 256], F16,
                                     isOutput=False)
    ident_d = nc.declare_dram_parameter("ident", [128, 128], F16,
                                        isOutput=False)
    # per-chain outputs: y{ch}[p, t, j*ncs + s] = g_t[u = j*128 + p, s]
    y_aps = [nc.declare_dram_parameter(f"y{ch}", [128, t_steps, 2 * ncs],
                                       F16, isOutput=True).ap()
             for ch, ncs in enumerate(chunks)]
    x_ap, wk_ap, wr_ap = x_d.ap(), wk_d.ap(), wr_d.ap()

    nch = len(chunks)
    offs = [sum(chunks[:i]) for i in range(nch)]
    Tanh = mybir.ActivationFunctionType.Tanh

    with ExitStack() as ctx:
        tc = ctx.enter_context(tile.TileContext(nc))
        const = ctx.enter_context(tc.tile_pool(name="const", bufs=1))
        x_sb = const.tile([128, t_steps * s_total], F16)
        wk_sb = const.tile([128, W * 256], F16)
        wr0_sb = const.tile([128, W * 256], F16)
        wr1_sb = const.tile([128, W * 256], F16)
        ident_sb = const.tile([128, 128], F16)
        junk = const.tile([128, 512], F16)
        # per-chain g rings: slot k at cols [k*2*ncs, (k+1)*2*ncs)
        rings = [const.tile([128, RING * 2 * chunks[ch]], F16,
                            name=f"gring{ch}") for ch in range(nch)]

        nc.sync.dma_start(wk_sb[:], wk_ap[:])
        nc.vector.memset(junk[:], 0.0)
        n_xgroups = (t_steps + XDMA_TGROUP - 1) // XDMA_TGROUP

        def x_dma(grp):
            lo = grp * XDMA_TGROUP * s_total
            hi = min((grp + 1) * XDMA_TGROUP, t_steps) * s_total
            nc.sync.dma_start(x_sb[:, lo:hi], x_ap[:, lo:hi])

        # first groups up front; the rest staggered inside the loop so the
        # output DMAs are not starved early (ring-reuse stalls drop the HAM
        # clock gate otherwise)
        for grp in range(min(XDMA_PRE, n_xgroups)):
            x_dma(grp)
        nc.sync.dma_start(wr0_sb[:], wr_ap[0:128, :])
        nc.sync.dma_start(wr1_sb[:], wr_ap[128:256, :])
        nc.sync.dma_start(ident_sb[:], ident_d.ap()[:])

        s16_pool = ctx.enter_context(tc.tile_pool(name="s16", bufs=2))
        # psum: chunks 0/1 get 3-deep rotation (6 banks); chunk 2 gets 2
        # (2 banks); the heater shares chunk 2's tag.
        ps_a = ctx.enter_context(tc.tile_pool(name="psa", bufs=3,
                                              space="PSUM"))
        ps_b = ctx.enter_context(tc.tile_pool(name="psb", bufs=2,
                                              space="PSUM"))

        def new_bank(ch, wi):
            pool = ps_a if ch < 2 else ps_b
            return pool.tile([128, 2 * chunks[ch]], F32, tag=f"ps{ch}",
                             name=f"ps{ch}_w{wi}")

        heat_ps = ps_b.tile([128, 2 * chunks[2]], F32, tag="ps2",
                            name="heat_ps")

        def heat(n, ps):
            for _ in range(n):
                nc.tensor.matmul(ps[:], junk[:, 0:128],
                                 junk[:, 0:2 * chunks[2]],
                                 start=True, stop=True)

        def gslot(ch, t):
            ncs = chunks[ch]
            k = t % RING
            return rings[ch][:, k * 2 * ncs:(k + 1) * 2 * ncs]

        # initial heater burst: warms HAM while x streams in
        heat(heat_burst, heat_ps)

        def x_mms(ps, ch, t, tau, start, stop_last=False):
            """xk'_t (10^tau-scaled): 2 K=128 matmuls vs 10^tau*[Wk;-.1Wk]."""
            ncs, off = chunks[ch], offs[ch]
            sl = slice(t * s_total + off, t * s_total + off + ncs)
            for j in range(2):
                nc.tensor.matmul(ps[:, j * ncs:(j + 1) * ncs],
                                 wk_sb[:, tau * 256 + j * 128:
                                       tau * 256 + (j + 1) * 128],
                                 x_sb[:, sl], start=start,
                                 stop=(stop_last and j == 1))

        def w_mms(ps, ch, g, tau, stop_last=True):
            """g @ (10^tau * Wr'): 4 K=128 matmuls."""
            ncs = chunks[ch]
            o = tau * 256
            nc.tensor.matmul(ps[:, 0:ncs], wr0_sb[:, o:o + 128],
                             g[:, 0:ncs], start=False, stop=False)
            nc.tensor.matmul(ps[:, 0:ncs], wr1_sb[:, o:o + 128],
                             g[:, ncs:2 * ncs], start=False, stop=False)
            nc.tensor.matmul(ps[:, ncs:2 * ncs], wr0_sb[:, o + 128:o + 256],
                             g[:, 0:ncs], start=False, stop=False)
            nc.tensor.matmul(ps[:, ncs:2 * ncs], wr1_sb[:, o + 128:o + 256],
                             g[:, ncs:2 * ncs], start=False, stop=stop_last)

        # prologue: window 0, tau=0: bank = xk_0 (x_{-1} rows are zero)
        cur = []
        for ch in range(nch):
            ps = new_bank(ch, 0)
            x_mms(ps, ch, 0, 0, start=True, stop_last=True)
            cur.append(ps)

        carry_scale = float(DECAY * 0.1 ** (W - 1))
        for t in range(t_steps):
            heat(heat_per_step, heat_ps)
            if t > 0 and t % XDMA_TGROUP == 0:
                grp = t // XDMA_TGROUP + XDMA_PRE - 1
                if grp < n_xgroups:
                    x_dma(grp)
            tau = t % W
            boundary = t + 1 < t_steps and (t + 1) % W == 0
            nxt = []
            for ch in range(nch):
                ncs = chunks[ch]
                ps = cur[ch]
                g = gslot(ch, t)
                if boundary:
                    # carry 0.1*z to sbuf BEFORE the tanh: same-tile reads
                    # serialize in emission order, so this way the identity
                    # inject + x matmuls overlap the tanh instead of
                    # following it.
                    s16 = s16_pool.tile([128, 2 * ncs], F16,
                                        tag=f"s16_{ch}",
                                        name=f"s16_{ch}_{t}")
                    nc.vector.tensor_scalar_mul(s16[:], ps[:], carry_scale)
                nc.scalar.activation(g[:], ps[:], Tanh,
                                     scale=float(0.1 ** tau))
                if t + 1 < t_steps:
                    if not boundary:
                        # continue accumulating in the same bank
                        tau2 = (t + 1) % W
                        x_mms(ps, ch, t + 1, tau2, start=False)
                        w_mms(ps, ch, g, tau2)
                        nxt.append(ps)
                    else:
                        ps2 = new_bank(ch, (t + 1) // W)
                        nc.tensor.matmul(ps2[:], ident_sb[:], s16[:],
                                         start=True, stop=False)
                        x_mms(ps2, ch, t + 1, 0, start=False)
                        w_mms(ps2, ch, g, 0)
                        nxt.append(ps2)

                last4 = t >= t_steps - 4
                batch = 2 if last4 else DMA_BATCH
                if t % batch == batch - 1:
                    # ring slots for the batch are contiguous, as is dst
                    k0 = (t - (batch - 1)) % RING
                    src = rings[ch][:, k0 * 2 * ncs:
                                    (k0 + batch) * 2 * ncs]
                    dst = y_aps[ch][:, t - (batch - 1):t + 1, :]
                    nc.sync.dma_start(dst, src)
            cur = nxt

    nc.compile()
    return nc


_PROGRAM = None

# test-harness knob: when trace=True, the BassKernelResults (with
# exec_time_ns from neuron-profile) is stashed in PROFILE["last"].
PROFILE = {"trace": False, "last": None}


def _get_program():
    global _PROGRAM
    if _PROGRAM is None:
        _PROGRAM = build_program()
    return _PROGRAM


def _pack_x(xs, t_steps, s_total):
    """(S, T, C) fp32 -> packed (128, T*S) fp16: rows 0-63 x_t, 64-127
    x_{t-1} (zeros at t=0)."""
    xt = np.ascontiguousarray(xs.transpose(2, 1, 0))      # (C, T, S)
    packed = np.zeros((128, t_steps * s_total), np.float16)
    pv = packed.reshape(2, 64, t_steps, s_total)
    pv[0] = xt
    pv[1, :, 1:] = xt[:, :-1]
    return packed.reshape(128, t_steps * s_total)


def kernel(**inputs):
    x = np.asarray(inputs["inputs"], np.float32)          # (8,128,128,64)
    wsets = [
        (np.asarray(inputs["h_fwd_k"]), np.asarray(inputs["h_fwd_r"])),
        (np.asarray(inputs["h_bwd_k"]), np.asarray(inputs["h_bwd_r"])),
        (np.asarray(inputs["v_fwd_k"]), np.asarray(inputs["v_fwd_r"])),
        (np.asarray(inputs["v_bwd_k"]), np.asarray(inputs["v_bwd_r"])),
    ]
    nc = _get_program()

    in_maps = []
    for core in range(N_CORES):
        scan, bhalf = core // 2, core % 2
        xb = x[bhalf * 4:(bhalf + 1) * 4]                 # (4, NH, NW, C)
        if scan >= 2:                                     # vertical: cols as seqs
            xb = xb.transpose(0, 2, 1, 3)                 # (4, NW, NH, C)
        xs = xb.reshape(S, T, C)
        if scan % 2 == 1:                                 # bwd: reverse time
            xs = np.ascontiguousarray(xs[:, ::-1])
        wk, wr = wsets[scan]
        wk2 = np.concatenate([wk, -DECAY * wk], axis=0)             # (128,256)
        wks = np.concatenate([(10.0 ** tau) * wk2 for tau in range(W)],
                             axis=1).astype(np.float16)             # (128,W*256)
        wrs = np.concatenate([(10.0 ** tau) * LEAKY * wr
                              for tau in range(W)],
                             axis=1).astype(np.float16)             # (256,W*256)
        in_maps.append({"x": _pack_x(xs, T, S), "wk": wks, "wr": wrs,
                        "ident": np.eye(128, dtype=np.float16)})

    res = run_bass_kernel_spmd(nc, in_maps, list(range(N_CORES)),
                               trace=PROFILE["trace"])
    PROFILE["last"] = res
    results = res.results

    out = np.empty((B, NH, NW, 4 * U), np.float32)
    for core in range(N_CORES):
        scan, bhalf = core // 2, core % 2
        # concat per-chain outputs (128, T, 2*ncs) back to (p, t, j, s)
        g = np.concatenate(
            [results[core][f"y{ch}"].reshape(128, T, 2, ncs)
             for ch, ncs in enumerate(CHUNKS)], axis=3).astype(np.float32)
        # host IIR: w_t = 0.1*w_{t-1} + g_t;  h = 0.9*w
        h = np.empty_like(g)
        w = np.zeros((128, 2, S), np.float32)
        for t in range(T):
            w = DECAY * w + g[:, t]
            h[:, t] = w
        h *= LEAKY
        hs = h.transpose(3, 1, 2, 0).reshape(S, T, U)     # (s, t, u=(j,p))
        if scan % 2 == 1:
            hs = hs[:, ::-1]
        dst = out[bhalf * 4:(bhalf + 1) * 4, :, :, scan * U:(scan + 1) * U]
        if scan < 2:
            dst[:] = hs.reshape(4, NH, NW, U)
        else:
            dst[:] = hs.reshape(4, NW, NH, U).transpose(0, 2, 1, 3)
    return out
